# revision 2
# baseline (speedup 1.0000x reference)
"""Deformable Conv2d (B=4, Cin=64, Cout=128, H=W=128, K=3) on 8 trn2 cores.

Sharding: data-parallel over (batch, H-half): core s -> image s//2,
rows [64*(s%2), +64). All FLOPs on device:
  - offset/modulator 3x3 convs on PE (pos-major out via x-as-lhsT)
  - bilinear corner weights + gather indices on DVE/ACT
  - 4-corner gather via SWDGE dma_gather (512B/descriptor, bf16),
    one 9-tap gather per 16-row block
  - idx wrap (pos-partition -> 16-partition-wrapped + 8x replicated)
    via a small p-major DRAM bounce with 288B-granule descriptors
    plus one DVE free-dim reorder (vs. 2-byte-granule scatter DMAs)
  - corner combine: one broadcast tensor_tensor multiply per (tap, xc)
  - corner-sum + transpose via PE transpose into PSUM
  - 576->128 einsum on PE (bf16, f32 PSUM)
Host side: input layout prep (padded shards, row-pair-duplicated gather
source, weight reordering, constant tables) and output reassembly.
"""

import numpy as np
import ml_dtypes

import concourse.bass as bass
import concourse.bacc as bacc
import concourse.mybir as mybir
from concourse.tile import TileContext
from concourse import library_config

F32 = mybir.dt.float32
BF16 = mybir.dt.bfloat16
I16 = mybir.dt.int16
I32 = mybir.dt.int32
ALU = mybir.AluOpType
_FLOOR_BIAS = -0.5  # HW float->int cast rounds; sim truncates (use 0.0)
ACTF = mybir.ActivationFunctionType

B, Cin, Cout, H, W = 4, 64, 128, 128, 128
Hs = 64                      # rows per shard
PADY = PADX = 4
Hp, Wp = 73, 136             # X2 padded dims
CONV_H, CONV_W = Hs + 2, W + 2   # 66 x 130 conv input (pad 1)
CONV_FLAT = CONV_H * CONV_W      # 8580
ELEM = 256                   # gather element: (xc2, c64, yc2) bf16 = 512B
HB = 16                      # rows per processing block
NBLK = Hs // HB              # 4 blocks
NPOS = HB * W                # 2048 positions per block
NIDX = 9 * NPOS              # 18432 gather idxs per block
TAPS = 9
# tap groups for the offset/modulator conv
# (pair groups use the +1-shifted second half of the x tile; K=128)
GROUPS = [(0, 1), (3, 4), (6, 7), (2,), (5,), (8,)]


def _conv_off(k, h):
    ky, kx = divmod(k, 3)
    return (h + ky) * CONV_W + kx


def build_nc():
    nc = bacc.Bacc("TRN2")

    xab = nc.dram_tensor("xab", [128, CONV_FLAT], BF16, kind="ExternalInput")
    # gather source: point rows of 128 (c, yc); an element spans two
    # consecutive points (xc) = 256 values, so rows overlap (elem_step=128)
    x2 = nc.dram_tensor("x2", [Hp * Wp + 1, 128], BF16, kind="ExternalInput")
    ck = nc.dram_tensor("ck", [128, 27], F32, kind="ExternalInput")
    chv = nc.dram_tensor("chv", [128, Hs], F32, kind="ExternalInput")
    pvec = nc.dram_tensor("pvec", [128, 1], F32, kind="ExternalInput")
    wconv = nc.dram_tensor("wconv", [128, 6, 27], BF16, kind="ExternalInput")
    # einsum weights: chunk per tap, rows = (c, yc) (yc-duplicated)
    wmain = nc.dram_tensor("wmain", [128, TAPS, 128], BF16,
                           kind="ExternalInput")
    ident = nc.dram_tensor("ident", [128, 128], BF16, kind="ExternalInput")
    out = nc.dram_tensor("out", [128, Hs * W], F32, kind="ExternalOutput")
    # idx bounce scratch, p-major [p, (k, h)]
    idxs_dram = nc.dram_tensor("idxs_scratch", [2, 128, TAPS * HB], I16,
                               kind="Internal")

    with TileContext(nc) as tc:
        with tc.tile_pool(name="static", bufs=1) as static, \
             tc.tile_pool(name="offp", bufs=2) as offp, \
             tc.tile_pool(name="fld", bufs=2) as fld, \
             tc.tile_pool(name="idxp", bufs=2) as idxp, \
             tc.tile_pool(name="vp", bufs=2) as vp, \
             tc.tile_pool(name="stp", bufs=1) as stp, \
             tc.tile_pool(name="outp", bufs=2) as outp, \
             tc.tile_pool(name="pconv", bufs=2, space="PSUM") as pconv, \
             tc.tile_pool(name="ptac", bufs=2, space="PSUM") as ptac, \
             tc.tile_pool(name="pout", bufs=2, space="PSUM") as pout:

            nc.gpsimd.load_library(library_config.mlp)

            # ---- static tiles ----
            t_xab = static.tile([128, CONV_FLAT], BF16)
            nc.sync.dma_start(t_xab[:], xab[:])
            t_wconv = static.tile([128, 6, 27], BF16)
            nc.sync.dma_start(t_wconv[:], wconv[:])
            t_wmain = static.tile([128, TAPS, 128], BF16)
            nc.sync.dma_start(t_wmain[:], wmain[:])
            t_ck = static.tile([128, 27], F32)
            nc.sync.dma_start(t_ck[:], ck[:])
            t_chv = static.tile([128, Hs], F32)
            nc.sync.dma_start(t_chv[:], chv[:])
            t_pvec = static.tile([128, 1], F32)
            nc.sync.dma_start(t_pvec[:], pvec[:])
            # identity for PE transpose (host-provided)
            t_ident = static.tile([128, 128], BF16)
            nc.sync.dma_start(t_ident[:], ident[:])

            for blk in range(NBLK):
                h0 = blk * HB
                # ---- conv: offsets+modulator, pos-major [128(w), h, 27] ----
                t_off = offp.tile([128, HB, 27], BF16, tag="off")
                for h in range(HB):
                    hg = h0 + h
                    ps = pconv.tile([128, 27], F32, tag="pc")
                    for j, grp in enumerate(GROUPS):
                        kdim = 64 if len(grp) == 1 else 128
                        o = _conv_off(grp[0], hg)
                        nc.tensor.matmul(
                            ps[:], t_xab[:kdim, o:o + W],
                            t_wconv[:kdim, j, :],
                            start=(j == 0), stop=(j == len(GROUPS) - 1))
                    nc.scalar.copy(t_off[:, h, :], ps[:])

                # ---- fields ----
                # F = off + ck[k] (+ h for y cols, + p for x cols)
                t_F = fld.tile([128, HB, 27], F32, tag="F")
                nc.vector.tensor_tensor(
                    t_F[:], t_off[:],
                    bass.AP(tensor=t_ck[:].tensor, offset=t_ck[:].offset,
                            ap=[list(t_ck[:].ap[0]), [0, HB], [1, 27]]),
                    ALU.add)
                chs = t_chv[:, h0:h0 + HB]
                nc.vector.tensor_tensor(
                    t_F[:, :, 0:9], t_F[:, :, 0:9],
                    bass.AP(tensor=chs.tensor, offset=chs.offset,
                            ap=[list(chs.ap[0]), [1, HB], [0, 9]]),
                    ALU.add)
                nc.vector.tensor_scalar(t_F[:, :, 9:18], t_F[:, :, 9:18],
                                        t_pvec[:], None, ALU.add)
                t_Fm = fld.tile([128, HB, 18], F32, tag="Fm")
                nc.vector.tensor_scalar(t_Fm[:], t_F[:, :, 0:18], _FLOOR_BIAS,
                                        None, ALU.add)
                t_i32 = fld.tile([128, HB, 18], I32, tag="i32")
                nc.vector.tensor_copy(t_i32[:], t_Fm[:])
                t_fl = fld.tile([128, HB, 18], F32, tag="fl")
                nc.vector.tensor_copy(t_fl[:], t_i32[:])
                t_fr = fld.tile([128, HB, 18], F32, tag="fr")
                nc.vector.tensor_tensor(t_fr[:], t_F[:, :, 0:18], t_fl[:],
                                        ALU.subtract)
                ty = t_fr[:, :, 0:9]
                tx = t_fr[:, :, 9:18]
                t_mask = fld.tile([128, HB, 9], F32, tag="mask")
                nc.scalar.activation(t_mask[:], t_F[:, :, 18:27], ACTF.Sigmoid)
                t_w11 = fld.tile([128, HB, 9], F32, tag="w11")
                nc.vector.tensor_tensor(t_w11[:], ty, tx, ALU.mult)
                t_w01 = fld.tile([128, HB, 9], F32, tag="w01")
                nc.vector.tensor_tensor(t_w01[:], tx, t_w11[:], ALU.subtract)
                t_w10 = fld.tile([128, HB, 9], F32, tag="w10")
                nc.vector.tensor_tensor(t_w10[:], ty, t_w11[:], ALU.subtract)
                t_omty = fld.tile([128, HB, 9], F32, tag="omty")
                nc.vector.tensor_scalar(t_omty[:], ty, -1.0, 1.0,
                                        ALU.mult, ALU.add)
                t_w00 = fld.tile([128, HB, 9], F32, tag="w00")
                nc.vector.tensor_tensor(t_w00[:], t_omty[:], t_w01[:],
                                        ALU.subtract)
                # wcomb[p, h, k, xc, yc] bf16, mask folded
                t_wc = fld.tile([128, HB, TAPS, 2, 2], BF16, tag="wc")
                for (xc, yc, tw) in ((0, 0, t_w00), (0, 1, t_w10),
                                     (1, 0, t_w01), (1, 1, t_w11)):
                    nc.vector.tensor_tensor(t_wc[:, :, :, xc, yc], tw[:],
                                            t_mask[:], ALU.mult)
                # linear idx = y0*Wp + x0 (f32 exact) -> int16, free (k, h)
                t_lin = fld.tile([128, HB, 9], F32, tag="lin")
                nc.vector.tensor_scalar(t_lin[:], t_fl[:, :, 0:9], float(Wp),
                                        None, ALU.mult)
                nc.vector.tensor_tensor(t_lin[:], t_lin[:], t_fl[:, :, 9:18],
                                        ALU.add)
                t_i16 = fld.tile([128, TAPS, HB], I16, tag="i16")
                nc.vector.tensor_copy(
                    t_i16[:].rearrange("p k h -> p h k"), t_lin[:])

                # ---- idx wrap: [p, (k,h)] -> [r (x8), (k,h,q)] ----
                # bounce to DRAM p-major (contiguous per partition), read
                # back per 16-partition group with (q,k,h) order (288B
                # granules), then DVE-reorder free dims to (k,h,q).
                base = idxs_dram[blk % 2]
                nc.sync.dma_start(
                    base, t_i16[:].rearrange("p k h -> p (k h)"))
                t_q = idxp.tile([128, TAPS * HB * 8], I16, tag="iq")
                kh = TAPS * HB
                for g in range(8):
                    src_view = bass.AP(
                        tensor=base.tensor, offset=base.offset,
                        ap=[[kh, 16], [16 * kh, 8], [1, kh]])
                    nc.sync.dma_start(t_q[16 * g:16 * (g + 1)], src_view)
                t_idxw = idxp.tile([128, TAPS * HB * 8], I16, tag="ix")
                nc.vector.tensor_copy(
                    t_idxw[:].rearrange("p (k h q) -> p k h q", h=HB, q=8),
                    t_q[:].rearrange("p (q k h) -> p k h q", q=8, k=TAPS))

                # ---- gather: one 9-tap SWDGE gather per block ----
                x2ap = x2[:]
                x2ov = bass.AP(tensor=x2ap.tensor, offset=x2ap.offset,
                               ap=[[128, Hp * Wp], [1, ELEM]])
                t_v = vp.tile([128, TAPS, HB, ELEM], BF16, tag="v")
                nc.gpsimd.dma_gather(
                    t_v[:].rearrange("p k h e -> p (k h) e"), x2ov,
                    t_idxw[:], NIDX, NIDX, ELEM, elem_step=128,
                    single_packet=False)

                # ---- per tap: combine, transpose ----
                t_st = stp.tile([128, TAPS, HB, 128], BF16, tag="st")
                wc = t_wc[:]
                for k in range(TAPS):
                    # combine: U = V * broadcast(wc), in place.
                    # per (tap, xc): ISA allows only 3 free dims
                    for xc in range(2):
                        vv = t_v[:, k, :, xc * 128:(xc + 1) * 128]
                        wv = bass.AP(
                            tensor=wc.tensor,
                            offset=wc.offset + k * 4 + xc * 2,
                            ap=[list(wc.ap[0]), [TAPS * 4, HB],
                                [0, 64], [1, 2]])
                        nc.vector.tensor_tensor(vv, vv, wv, ALU.mult)
                    # x-corner sum into the x0 half (DVE; PSUM cannot
                    # accumulate bf16 transposes on hw)
                    nc.vector.tensor_tensor(
                        t_v[:, k, :, 0:128], t_v[:, k, :, 0:128],
                        t_v[:, k, :, 128:256], ALU.add)
                    # transpose into psum; rows become (c, yc);
                    # y-corner sum happens in einsum (duplicated W rows)
                    vap = t_v[:]
                    for g in range(HB // 8):
                        pt = ptac.tile([128, 8, 128], BF16, tag="pt")
                        for h in range(8):
                            hloc = g * 8 + h
                            uap = bass.AP(
                                tensor=vap.tensor,
                                offset=vap.offset + k * HB * ELEM
                                + hloc * ELEM,
                                ap=[list(vap.ap[0]), [1, 128]])
                            nc.tensor.matmul(
                                pt[:, h, :], uap, t_ident[:],
                                start=True, stop=True,
                                is_transpose=True)
                        nc.scalar.copy(
                            t_st[:, k, g * 8:(g + 1) * 8, :], pt[:])

                # ---- einsum: out[o, pos] += wmain_k^T @ S_T_k ----
                for cc in range(NPOS // 512):
                    po = pout.tile([128, 512], F32, tag="po")
                    for j in range(TAPS):
                        stv = t_st[:, j].rearrange("p h w -> p (h w)")
                        nc.tensor.matmul(
                            po[:], t_wmain[:, j, :],
                            stv[:, cc * 512:(cc + 1) * 512],
                            start=(j == 0), stop=(j == TAPS - 1))
                    t_out = outp.tile([128, 512], F32, tag="out")
                    nc.scalar.copy(t_out[:], po[:])
                    base_o = blk * NPOS + cc * 512
                    nc.sync.dma_start(
                        out[:, base_o:base_o + 512], t_out[:])

    nc.finalize()
    return nc


# ---------------- host side ----------------

def prep_core_inputs(x, w_off, b_off, w_mod, b_mod, w_reg, s):
    """Build device input dict for shard s (image s//2, rows 64*(s%2)+)."""
    b, half = divmod(s, 2)
    r0 = half * Hs
    xb = np.asarray(x[b], dtype=np.float32)           # [C, H, W]

    # conv input, channel-major padded [64, 66, 130] bf16; second partition
    # half is the same data shifted by +1 element (for pair tap groups)
    xcm = np.zeros((Cin, CONV_H, CONV_W), np.float32)
    ylo = r0 - 1
    sylo, syhi = max(ylo, 0), min(ylo + CONV_H, H)
    xcm[:, sylo - ylo:syhi - ylo, 1:1 + W] = xb[:, sylo:syhi, :]
    xf = xcm.reshape(Cin, CONV_FLAT).astype(ml_dtypes.bfloat16)
    xab = np.zeros((128, CONV_FLAT), ml_dtypes.bfloat16)
    xab[:Cin] = xf
    xab[Cin:, :-1] = xf[:, 1:]

    # X2 gather source: [Hp, Wp, Cin, 2(yc)] -> flat [Hp*Wp (+1), 128]
    X2 = np.zeros((Hp, Wp, Cin, 2), np.float32)
    for yc in range(2):
        ylo = r0 - PADY + yc
        sylo, syhi = max(ylo, 0), min(ylo + Hp, H)
        X2[sylo - ylo:syhi - ylo, PADX:PADX + W, :, yc] = \
            xb[:, sylo:syhi, :].transpose(1, 2, 0)
    x2 = np.zeros((Hp * Wp + 1, Cin * 2), ml_dtypes.bfloat16)
    x2[:Hp * Wp] = X2.reshape(Hp * Wp, Cin * 2).astype(ml_dtypes.bfloat16)

    # decomposed consts: ck[*, 27] per-tap, chv[*, h]=h, pvec[p,1]=p
    ck = np.zeros((128, 27), np.float32)
    for k in range(TAPS):
        ky, kx = divmod(k, 3)
        ck[:, k] = ky + (PADY - 1) + float(b_off[2 * k])
        ck[:, 9 + k] = kx + (PADX - 1) + float(b_off[2 * k + 1])
        ck[:, 18 + k] = float(b_mod[k])
    chv = np.broadcast_to(np.arange(Hs, dtype=np.float32)[None, :],
                          (128, Hs)).copy()
    pvec = np.arange(128, dtype=np.float32).reshape(128, 1).copy()

    # conv weights [128, 6, 27]: group rows = [c of k0 | c of k1]
    wconv = np.zeros((128, 6, 27), np.float32)
    for j, grp in enumerate(GROUPS):
        for t, k in enumerate(grp):
            ky, kx = divmod(k, 3)
            for o in range(9):
                wconv[t * 64:(t + 1) * 64, j, o] = w_off[2 * o, :, ky, kx]
                wconv[t * 64:(t + 1) * 64, j, 9 + o] = w_off[2 * o + 1, :, ky, kx]
                wconv[t * 64:(t + 1) * 64, j, 18 + o] = w_mod[o, :, ky, kx]
    wconv = wconv.astype(ml_dtypes.bfloat16)

    # main weights [128, 9, 128]: chunk k rows = (c, yc) duplicated
    wmain = np.zeros((128, TAPS, 128), np.float32)
    for k in range(TAPS):
        ky, kx = divmod(k, 3)
        wt = 2.0 * w_reg[:, :, ky, kx].T        # [c, o]
        wmain[0::2, k, :] = wt
        wmain[1::2, k, :] = wt
    wmain = wmain.astype(ml_dtypes.bfloat16)

    return {"xab": xab, "x2": x2, "ck": ck, "chv": chv, "pvec": pvec,
            "wconv": wconv, "wmain": wmain,
            "ident": np.eye(128, dtype=ml_dtypes.bfloat16)}


_NC_CACHE = {}


def _run(x, w_off, b_off, w_mod, b_mod, w_reg, trace=False, **spmd_kwargs):
    from concourse.bass_utils import run_bass_kernel_spmd
    x = np.asarray(x); w_off = np.asarray(w_off); b_off = np.asarray(b_off)
    w_mod = np.asarray(w_mod); b_mod = np.asarray(b_mod)
    w_reg = np.asarray(w_reg)

    if "nc" not in _NC_CACHE:
        _NC_CACHE["nc"] = build_nc()
    nc = _NC_CACHE["nc"]

    in_maps = [prep_core_inputs(x, w_off, b_off, w_mod, b_mod, w_reg, s)
               for s in range(8)]
    res = run_bass_kernel_spmd(nc, in_maps, core_ids=list(range(8)),
                               trace=trace, **spmd_kwargs)
    results = res.results if hasattr(res, "results") else res

    out = np.zeros((B, Cout, H, W), np.float32)
    for s in range(8):
        b, half = divmod(s, 2)
        r0 = half * Hs
        out[b, :, r0:r0 + Hs, :] = \
            np.asarray(results[s]["out"]).reshape(Cout, Hs, W)
    return out, res


def kernel(x, w_off, b_off, w_mod, b_mod, w_reg):
    out, _ = _run(x, w_off, b_off, w_mod, b_mod, w_reg)
    return out


# revision 5
# speedup vs baseline: 4.4689x; 4.4689x over previous
"""Deformable Conv2d (B=4, Cin=64, Cout=128, H=W=128, K=3) on 8 trn2 cores.

Sharding: data-parallel over (batch, H-half): core s -> image s//2,
rows [64*(s%2), +64). All FLOPs on device:
  - offset/modulator 3x3 convs on PE (pos-major out via x-as-lhsT)
  - bilinear corner weights + gather indices on DVE/ACT
  - 4-corner gather via SWDGE dma_gather (512B/descriptor, bf16),
    split across 4 SWDGE queues per 8-row block so all four Q7 core
    pairs generate descriptors concurrently (desc-gen is the gather
    bottleneck at ~9.5ns/idx on one core pair)
  - idx wrap (pos-partition -> 16-partition-wrapped + 8x replicated)
    via a small p-major DRAM bounce with 144B-granule descriptors
    plus one DVE free-dim reorder (vs. 2-byte-granule scatter DMAs)
  - corner combine: one broadcast tensor_tensor multiply per (tap, xc)
  - corner-sum + transpose via PE transpose into PSUM
  - 576->128 einsum on PE (bf16, f32 PSUM)
Small per-block tensors are packed into a few large SBUF tiles with
bitcast views (tile slots pad to 4KB each).
Host side: input layout prep (padded shards, row-pair-duplicated gather
source, weight reordering, constant tables) and output reassembly.
"""

import numpy as np
import ml_dtypes

import concourse.bass as bass
import concourse.bacc as bacc
import concourse.mybir as mybir
from concourse.tile import TileContext
from concourse import library_config

F32 = mybir.dt.float32
BF16 = mybir.dt.bfloat16
I16 = mybir.dt.int16
I32 = mybir.dt.int32
ALU = mybir.AluOpType
_FLOOR_BIAS = -0.5  # HW float->int cast rounds; sim truncates (use 0.0)
ACTF = mybir.ActivationFunctionType

B, Cin, Cout, H, W = 4, 64, 128, 128, 128
Hs = 64                      # rows per shard
PADY = PADX = 4
Hp, Wp = 73, 136             # X2 padded dims
CONV_H, CONV_W = Hs + 2, W + 2   # 66 x 130 conv input (pad 1)
CONV_FLAT = CONV_H * CONV_W      # 8580
ELEM = 256                   # gather element: (xc2, c64, yc2) bf16 = 512B
HB = 8                       # rows per processing block
NBLK = Hs // HB              # 8 blocks
NPOS = HB * W                # 1024 positions per block
NIDX = 9 * NPOS              # 9216 gather idxs per block
NQ = 4                       # SWDGE queues (Q7 core pairs)
TAPS = 9
KH = TAPS * HB               # 72 (k, h) idx slots per position-partition
# tap groups for the offset/modulator conv
# (pair groups use the +1-shifted second half of the x tile; K=128)
GROUPS = [(0, 1), (3, 4), (6, 7), (2,), (5,), (8,)]


def _conv_off(k, h):
    ky, kx = divmod(k, 3)
    return (h + ky) * CONV_W + kx


def build_nc():
    nc = bacc.Bacc("TRN2", num_swdge_queues=NQ)

    xab = nc.dram_tensor("xab", [128, CONV_FLAT], BF16, kind="ExternalInput")
    # gather source: point rows of 128 (c, yc); an element spans two
    # consecutive points (xc) = 256 values, so rows overlap (elem_step=128)
    x2 = nc.dram_tensor("x2", [Hp * Wp + 1, 128], BF16, kind="ExternalInput")
    ck = nc.dram_tensor("ck", [128, 27], F32, kind="ExternalInput")
    chv = nc.dram_tensor("chv", [128, Hs], F32, kind="ExternalInput")
    pvec = nc.dram_tensor("pvec", [128, 1], F32, kind="ExternalInput")
    wconv = nc.dram_tensor("wconv", [128, 6, 27], BF16, kind="ExternalInput")
    # einsum weights: chunk per tap, rows = (c, yc) (yc-duplicated)
    wmain = nc.dram_tensor("wmain", [128, TAPS, 128], BF16,
                           kind="ExternalInput")
    ident = nc.dram_tensor("ident", [128, 128], BF16, kind="ExternalInput")
    out = nc.dram_tensor("out", [128, Hs * W], F32, kind="ExternalOutput")
    # idx bounce scratch, p-major [p, (k, h)]
    idxs_dram = nc.dram_tensor("idxs_scratch", [2, 128, KH], I16,
                               kind="Internal")

    with TileContext(nc) as tc:
        with tc.tile_pool(name="static", bufs=1) as static, \
             tc.tile_pool(name="flda", bufs=2) as flda, \
             tc.tile_pool(name="fldb", bufs=1) as fldb, \
             tc.tile_pool(name="idxp", bufs=2) as idxp, \
             tc.tile_pool(name="vp", bufs=2) as vp, \
             tc.tile_pool(name="stp", bufs=1) as stp, \
             tc.tile_pool(name="outp", bufs=2) as outp, \
             tc.tile_pool(name="pconv", bufs=2, space="PSUM") as pconv, \
             tc.tile_pool(name="ptac", bufs=2, space="PSUM") as ptac, \
             tc.tile_pool(name="pout", bufs=2, space="PSUM") as pout:

            nc.gpsimd.load_library(library_config.mlp)

            # ---- static tiles (bf16 + f32 consts packed) ----
            t_xab = static.tile([128, CONV_FLAT], BF16)
            nc.sync.dma_start(t_xab[:], xab[:])
            t_sb = static.tile([128, 1442], BF16)   # wconv|wmain|ident
            v_wconv = t_sb[:, 0:162].rearrange("p (g o) -> p g o", g=6)
            v_wmain = t_sb[:, 162:1314].rearrange("p (k o) -> p k o", k=TAPS)
            v_ident = t_sb[:, 1314:1442]
            nc.sync.dma_start(v_wconv, wconv[:])
            nc.sync.dma_start(v_wmain, wmain[:])
            nc.sync.dma_start(v_ident, ident[:])
            t_sf = static.tile([128, 92], F32)      # ck|chv|pvec
            v_ck = t_sf[:, 0:27]
            v_chv = t_sf[:, 27:91]
            v_pvec = t_sf[:, 91:92]
            nc.sync.dma_start(v_ck, ck[:])
            nc.sync.dma_start(v_chv, chv[:])
            nc.sync.dma_start(v_pvec, pvec[:])

            x2ap = x2[:]
            x2ov = bass.AP(tensor=x2ap.tensor, offset=x2ap.offset,
                           ap=[[128, Hp * Wp], [1, ELEM]])

            for blk in range(NBLK):
                h0 = blk * HB
                # packed per-block bf16 tile: off | wc | i16
                t_fa = flda.tile([128, 576], BF16, tag="fa")
                v_off = t_fa[:, 0:216].rearrange("p (h k) -> p h k", h=HB)
                v_wc = t_fa[:, 216:504].rearrange(
                    "p (h k x y) -> p h k x y", h=HB, k=TAPS, x=2)
                v_i16 = t_fa[:, 504:576].bitcast(I16).rearrange(
                    "p (k h) -> p k h", k=TAPS)
                # packed per-block f32 scratch
                t_fb = fldb.tile([128, 1296], F32, tag="fb")
                v_F = t_fb[:, 0:216].rearrange("p (h k) -> p h k", h=HB)
                v_Fm = t_fb[:, 216:360].rearrange("p (h k) -> p h k", h=HB)
                v_i32 = t_fb[:, 360:504].bitcast(I32).rearrange(
                    "p (h k) -> p h k", h=HB)
                v_fl = t_fb[:, 504:648].rearrange("p (h k) -> p h k", h=HB)
                v_fr = t_fb[:, 648:792].rearrange("p (h k) -> p h k", h=HB)
                v_mask = t_fb[:, 792:864].rearrange("p (h k) -> p h k", h=HB)
                v_w11 = t_fb[:, 864:936].rearrange("p (h k) -> p h k", h=HB)
                v_w01 = t_fb[:, 936:1008].rearrange("p (h k) -> p h k", h=HB)
                v_w10 = t_fb[:, 1008:1080].rearrange("p (h k) -> p h k", h=HB)
                v_omty = t_fb[:, 1080:1152].rearrange("p (h k) -> p h k", h=HB)
                v_w00 = t_fb[:, 1152:1224].rearrange("p (h k) -> p h k", h=HB)
                v_lin = t_fb[:, 1224:1296].rearrange("p (h k) -> p h k", h=HB)

                # ---- conv: offsets+modulator, pos-major [128(w), h, 27] ----
                for h in range(HB):
                    hg = h0 + h
                    ps = pconv.tile([128, 27], F32, tag="pc")
                    for j, grp in enumerate(GROUPS):
                        kdim = 64 if len(grp) == 1 else 128
                        o = _conv_off(grp[0], hg)
                        nc.tensor.matmul(
                            ps[:], t_xab[:kdim, o:o + W],
                            v_wconv[:kdim, j, :],
                            start=(j == 0), stop=(j == len(GROUPS) - 1))
                    nc.scalar.copy(v_off[:, h, :], ps[:])

                # ---- fields ----
                # F = off + ck[k] (+ h for y cols, + p for x cols)
                nc.vector.tensor_tensor(
                    v_F, v_off,
                    bass.AP(tensor=v_ck.tensor, offset=v_ck.offset,
                            ap=[list(v_ck.ap[0]), [0, HB], [1, 27]]),
                    ALU.add)
                chs = v_chv[:, h0:h0 + HB]
                nc.vector.tensor_tensor(
                    v_F[:, :, 0:9], v_F[:, :, 0:9],
                    bass.AP(tensor=chs.tensor, offset=chs.offset,
                            ap=[list(chs.ap[0]), [1, HB], [0, 9]]),
                    ALU.add)
                nc.vector.tensor_scalar(v_F[:, :, 9:18], v_F[:, :, 9:18],
                                        v_pvec, None, ALU.add)
                nc.vector.tensor_scalar(v_Fm, v_F[:, :, 0:18], _FLOOR_BIAS,
                                        None, ALU.add)
                nc.vector.tensor_copy(v_i32, v_Fm)
                nc.vector.tensor_copy(v_fl, v_i32)
                nc.vector.tensor_tensor(v_fr, v_F[:, :, 0:18], v_fl,
                                        ALU.subtract)
                ty = v_fr[:, :, 0:9]
                tx = v_fr[:, :, 9:18]
                nc.scalar.activation(v_mask, v_F[:, :, 18:27], ACTF.Sigmoid)
                nc.vector.tensor_tensor(v_w11, ty, tx, ALU.mult)
                nc.vector.tensor_tensor(v_w01, tx, v_w11, ALU.subtract)
                nc.vector.tensor_tensor(v_w10, ty, v_w11, ALU.subtract)
                nc.vector.tensor_scalar(v_omty, ty, -1.0, 1.0,
                                        ALU.mult, ALU.add)
                nc.vector.tensor_tensor(v_w00, v_omty, v_w01, ALU.subtract)
                # wcomb[p, h, k, xc, yc] bf16, mask folded
                for (xc, yc, tw) in ((0, 0, v_w00), (0, 1, v_w10),
                                     (1, 0, v_w01), (1, 1, v_w11)):
                    nc.vector.tensor_tensor(v_wc[:, :, :, xc, yc], tw,
                                            v_mask, ALU.mult)
                # linear idx = y0*Wp + x0 (f32 exact) -> int16, free (k, h)
                nc.vector.tensor_scalar(v_lin, v_fl[:, :, 0:9], float(Wp),
                                        None, ALU.mult)
                nc.vector.tensor_tensor(v_lin, v_lin, v_fl[:, :, 9:18],
                                        ALU.add)
                nc.vector.tensor_copy(v_i16.rearrange("p k h -> p h k"),
                                      v_lin)

                # ---- idx wrap: [p, (k,h)] -> [r (x8), (k,h,q)] ----
                # bounce to DRAM p-major (contiguous per partition), read
                # back per 16-partition group with (q,k,h) order (144B
                # granules), then DVE-reorder free dims to (k,h,q).
                base = idxs_dram[blk % 2]
                nc.sync.dma_start(base, v_i16.rearrange("p k h -> p (k h)"))
                t_q = idxp.tile([128, KH * 8], I16, tag="iq")
                for g in range(8):
                    src_view = bass.AP(
                        tensor=base.tensor, offset=base.offset,
                        ap=[[KH, 16], [16 * KH, 8], [1, KH]])
                    nc.sync.dma_start(t_q[16 * g:16 * (g + 1)], src_view)
                t_idxw = idxp.tile([128, KH * 8], I16, tag="ix")
                nc.vector.tensor_copy(
                    t_idxw[:].rearrange("p (k h q) -> p k h q", h=HB, q=8),
                    t_q[:].rearrange("p (q k h) -> p k h q", q=8, k=TAPS))

                # ---- gather: 4 SWDGE queues (one per Q7 core pair) ----
                t_v = vp.tile([128, TAPS, HB, ELEM], BF16, tag="v")
                t_v_flat = t_v[:].rearrange("p k h e -> p (k h) e")
                per_q = KH // NQ                   # 18 dst slots of 128 idx
                for qi in range(NQ):
                    nidx_q = per_q * 128
                    nc.gpsimd.dma_gather(
                        t_v_flat[:, qi * per_q:(qi + 1) * per_q, :], x2ov,
                        t_idxw[:, qi * per_q * 8:(qi + 1) * per_q * 8],
                        nidx_q, nidx_q, ELEM, elem_step=128,
                        single_packet=False, queue_num=qi)

                # ---- per tap: combine, transpose ----
                t_st = stp.tile([128, TAPS, HB, 128], BF16, tag="st")
                for k in range(TAPS):
                    # combine: U = V * broadcast(wc), in place.
                    # per (tap, xc): ISA allows only 3 free dims
                    for xc in range(2):
                        vv = t_v[:, k, :, xc * 128:(xc + 1) * 128]
                        wv = bass.AP(
                            tensor=v_wc.tensor,
                            offset=v_wc.offset + k * 4 + xc * 2,
                            ap=[list(v_wc.ap[0]), [TAPS * 4, HB],
                                [0, 64], [1, 2]])
                        nc.vector.tensor_tensor(vv, vv, wv, ALU.mult)
                    # x-corner sum into the x0 half (DVE; PSUM cannot
                    # accumulate bf16 transposes on hw)
                    nc.vector.tensor_tensor(
                        t_v[:, k, :, 0:128], t_v[:, k, :, 0:128],
                        t_v[:, k, :, 128:256], ALU.add)
                    # transpose into psum; rows become (c, yc);
                    # y-corner sum happens in einsum (duplicated W rows)
                    vap = t_v[:]
                    pt = ptac.tile([128, HB, 128], BF16, tag="pt")
                    for h in range(HB):
                        uap = bass.AP(
                            tensor=vap.tensor,
                            offset=vap.offset + k * HB * ELEM + h * ELEM,
                            ap=[list(vap.ap[0]), [1, 128]])
                        nc.tensor.matmul(
                            pt[:, h, :], uap, v_ident,
                            start=True, stop=True, is_transpose=True)
                    nc.scalar.copy(t_st[:, k], pt[:])

                # ---- einsum: out[o, pos] += wmain_k^T @ S_T_k ----
                for cc in range(NPOS // 512):
                    po = pout.tile([128, 512], F32, tag="po")
                    for j in range(TAPS):
                        stv = t_st[:, j].rearrange("p h w -> p (h w)")
                        nc.tensor.matmul(
                            po[:], v_wmain[:, j, :],
                            stv[:, cc * 512:(cc + 1) * 512],
                            start=(j == 0), stop=(j == TAPS - 1))
                    t_out = outp.tile([128, 512], F32, tag="out")
                    nc.scalar.copy(t_out[:], po[:])
                    base_o = blk * NPOS + cc * 512
                    nc.sync.dma_start(
                        out[:, base_o:base_o + 512], t_out[:])

    nc.finalize()
    return nc


# ---------------- host side ----------------

def prep_core_inputs(x, w_off, b_off, w_mod, b_mod, w_reg, s):
    """Build device input dict for shard s (image s//2, rows 64*(s%2)+)."""
    b, half = divmod(s, 2)
    r0 = half * Hs
    xb = np.asarray(x[b], dtype=np.float32)           # [C, H, W]

    # conv input, channel-major padded [64, 66, 130] bf16; second partition
    # half is the same data shifted by +1 element (for pair tap groups)
    xcm = np.zeros((Cin, CONV_H, CONV_W), np.float32)
    ylo = r0 - 1
    sylo, syhi = max(ylo, 0), min(ylo + CONV_H, H)
    xcm[:, sylo - ylo:syhi - ylo, 1:1 + W] = xb[:, sylo:syhi, :]
    xf = xcm.reshape(Cin, CONV_FLAT).astype(ml_dtypes.bfloat16)
    xab = np.zeros((128, CONV_FLAT), ml_dtypes.bfloat16)
    xab[:Cin] = xf
    xab[Cin:, :-1] = xf[:, 1:]

    # X2 gather source: [Hp, Wp, Cin, 2(yc)] -> flat [Hp*Wp (+1), 128]
    X2 = np.zeros((Hp, Wp, Cin, 2), np.float32)
    for yc in range(2):
        ylo = r0 - PADY + yc
        sylo, syhi = max(ylo, 0), min(ylo + Hp, H)
        X2[sylo - ylo:syhi - ylo, PADX:PADX + W, :, yc] = \
            xb[:, sylo:syhi, :].transpose(1, 2, 0)
    x2 = np.zeros((Hp * Wp + 1, Cin * 2), ml_dtypes.bfloat16)
    x2[:Hp * Wp] = X2.reshape(Hp * Wp, Cin * 2).astype(ml_dtypes.bfloat16)

    # decomposed consts: ck[*, 27] per-tap, chv[*, h]=h, pvec[p,1]=p
    ck = np.zeros((128, 27), np.float32)
    for k in range(TAPS):
        ky, kx = divmod(k, 3)
        ck[:, k] = ky + (PADY - 1) + float(b_off[2 * k])
        ck[:, 9 + k] = kx + (PADX - 1) + float(b_off[2 * k + 1])
        ck[:, 18 + k] = float(b_mod[k])
    chv = np.broadcast_to(np.arange(Hs, dtype=np.float32)[None, :],
                          (128, Hs)).copy()
    pvec = np.arange(128, dtype=np.float32).reshape(128, 1).copy()

    # conv weights [128, 6, 27]: group rows = [c of k0 | c of k1]
    wconv = np.zeros((128, 6, 27), np.float32)
    for j, grp in enumerate(GROUPS):
        for t, k in enumerate(grp):
            ky, kx = divmod(k, 3)
            for o in range(9):
                wconv[t * 64:(t + 1) * 64, j, o] = w_off[2 * o, :, ky, kx]
                wconv[t * 64:(t + 1) * 64, j, 9 + o] = w_off[2 * o + 1, :, ky, kx]
                wconv[t * 64:(t + 1) * 64, j, 18 + o] = w_mod[o, :, ky, kx]
    wconv = wconv.astype(ml_dtypes.bfloat16)

    # main weights [128, 9, 128]: chunk k rows = (c, yc) duplicated
    wmain = np.zeros((128, TAPS, 128), np.float32)
    for k in range(TAPS):
        ky, kx = divmod(k, 3)
        wt = 2.0 * w_reg[:, :, ky, kx].T        # [c, o]
        wmain[0::2, k, :] = wt
        wmain[1::2, k, :] = wt
    wmain = wmain.astype(ml_dtypes.bfloat16)

    return {"xab": xab, "x2": x2, "ck": ck, "chv": chv, "pvec": pvec,
            "wconv": wconv, "wmain": wmain,
            "ident": np.eye(128, dtype=ml_dtypes.bfloat16)}


_NC_CACHE = {}


def _run(x, w_off, b_off, w_mod, b_mod, w_reg, trace=False, **spmd_kwargs):
    from concourse.bass_utils import run_bass_kernel_spmd
    x = np.asarray(x); w_off = np.asarray(w_off); b_off = np.asarray(b_off)
    w_mod = np.asarray(w_mod); b_mod = np.asarray(b_mod)
    w_reg = np.asarray(w_reg)

    if "nc" not in _NC_CACHE:
        _NC_CACHE["nc"] = build_nc()
    nc = _NC_CACHE["nc"]

    in_maps = [prep_core_inputs(x, w_off, b_off, w_mod, b_mod, w_reg, s)
               for s in range(8)]
    res = run_bass_kernel_spmd(nc, in_maps, core_ids=list(range(8)),
                               trace=trace, **spmd_kwargs)
    results = res.results if hasattr(res, "results") else res

    out = np.zeros((B, Cout, H, W), np.float32)
    for s in range(8):
        b, half = divmod(s, 2)
        r0 = half * Hs
        out[b, :, r0:r0 + Hs, :] = \
            np.asarray(results[s]["out"]).reshape(Cout, Hs, W)
    return out, res


def kernel(x, w_off, b_off, w_mod, b_mod, w_reg):
    out, _ = _run(x, w_off, b_off, w_mod, b_mod, w_reg)
    return out


# revision 11
# speedup vs baseline: 5.1139x; 1.1443x over previous
"""Deformable Conv2d (B=4, Cin=64, Cout=128, H=W=128, K=3) on 8 trn2 cores.

Sharding: data-parallel over (batch, H-half): core s -> image s//2,
rows [64*(s%2), +64). All FLOPs on device:
  - offset/modulator 3x3 convs on PE (pos-major out via x-as-lhsT)
  - bilinear corner weights + gather indices on DVE/ACT
  - 4-corner gather via SWDGE dma_gather (512B/descriptor, bf16),
    split across 4 SWDGE queues per 8-row block so all four Q7 core
    pairs generate descriptors concurrently (desc-gen is the gather
    bottleneck at ~9.5ns/idx on one core pair)
  - idx wrap (pos-partition -> 16-partition-wrapped + 8x replicated)
    via a small p-major DRAM bounce with 144B-granule descriptors
    plus one DVE free-dim reorder (vs. 2-byte-granule scatter DMAs)
  - corner combine: one broadcast tensor_tensor multiply per (tap, xc)
  - corner-sum + transpose via PE transpose into PSUM
  - 576->128 einsum on PE (bf16, f32 PSUM)
Small per-block tensors are packed into a few large SBUF tiles with
bitcast views (tile slots pad to 4KB each).
Host side: input layout prep (padded shards, row-pair-duplicated gather
source, weight reordering, constant tables) and output reassembly.
"""

import numpy as np
import ml_dtypes

import concourse.bass as bass
import concourse.bacc as bacc
import concourse.mybir as mybir
from concourse.tile import TileContext
from concourse import library_config

F32 = mybir.dt.float32
BF16 = mybir.dt.bfloat16
I16 = mybir.dt.int16
I32 = mybir.dt.int32
ALU = mybir.AluOpType
_FLOOR_BIAS = -0.5  # HW float->int cast rounds; sim truncates (use 0.0)
ACTF = mybir.ActivationFunctionType

B, Cin, Cout, H, W = 4, 64, 128, 128, 128
Hs = 64                      # rows per shard
PADY = PADX = 4
Hp, Wp = 73, 136             # X2 padded dims
CONV_H, CONV_W = Hs + 2, W + 2   # 66 x 130 conv input (pad 1)
CONV_FLAT = CONV_H * CONV_W      # 8580
ELEM = 256                   # gather element: (xc2, c64, yc2) bf16 = 512B
HB = 8                       # rows per processing block
NBLK = Hs // HB              # 8 blocks
NPOS = HB * W                # 1024 positions per block
NIDX = 9 * NPOS              # 9216 gather idxs per block
NQ = 4                       # SWDGE queues (Q7 core pairs)
TAPS = 9
KH = TAPS * HB               # 72 (k, h) idx slots per position-partition
# tap groups for the offset/modulator conv
# (pair groups use the +1-shifted second half of the x tile; K=128)
GROUPS = [(0, 1), (3, 4), (6, 7), (2,), (5,), (8,)]


def _conv_off(k, h):
    ky, kx = divmod(k, 3)
    return (h + ky) * CONV_W + kx


def build_nc():
    nc = bacc.Bacc("TRN2", num_swdge_queues=NQ)

    xab = nc.dram_tensor("xab", [128, CONV_FLAT], BF16, kind="ExternalInput")
    # gather source: point rows of 128 (c, yc); an element spans two
    # consecutive points (xc) = 256 values, so rows overlap (elem_step=128)
    x2 = nc.dram_tensor("x2", [Hp * Wp + 1, 128], BF16, kind="ExternalInput")
    ck = nc.dram_tensor("ck", [128, 27], F32, kind="ExternalInput")
    chv = nc.dram_tensor("chv", [128, Hs], F32, kind="ExternalInput")
    pvec = nc.dram_tensor("pvec", [128, 1], F32, kind="ExternalInput")
    wconv = nc.dram_tensor("wconv", [128, 6, 27], BF16, kind="ExternalInput")
    # einsum weights: chunk per tap, rows = (c, yc) (yc-duplicated)
    wmain = nc.dram_tensor("wmain", [128, TAPS, 128], BF16,
                           kind="ExternalInput")
    ident = nc.dram_tensor("ident", [128, 128], BF16, kind="ExternalInput")
    out = nc.dram_tensor("out", [128, Hs * W], F32, kind="ExternalOutput")
    # idx bounce scratch, p-major [p, (k, h)]
    idxs_dram = nc.dram_tensor("idxs_scratch", [2, 128, KH], I16,
                               kind="Internal")

    with TileContext(nc) as tc:
        with tc.tile_pool(name="static", bufs=1) as static, \
             tc.tile_pool(name="flda", bufs=2) as flda, \
             tc.tile_pool(name="fldb", bufs=1) as fldb, \
             tc.tile_pool(name="idxp", bufs=2) as idxp, \
             tc.tile_pool(name="vp", bufs=2) as vp, \
             tc.tile_pool(name="stp", bufs=1) as stp, \
             tc.tile_pool(name="outp", bufs=2) as outp, \
             tc.tile_pool(name="pconv", bufs=2, space="PSUM") as pconv, \
             tc.tile_pool(name="ptac", bufs=2, space="PSUM") as ptac, \
             tc.tile_pool(name="pout", bufs=2, space="PSUM") as pout:

            nc.gpsimd.load_library(library_config.mlp)

            # ---- static tiles (bf16 + f32 consts packed) ----
            t_xab = static.tile([128, CONV_FLAT], BF16)
            nc.sync.dma_start(t_xab[:], xab[:])
            t_sb = static.tile([128, 1442], BF16)   # wconv|wmain|ident
            v_wconv = t_sb[:, 0:162].rearrange("p (g o) -> p g o", g=6)
            v_wmain = t_sb[:, 162:1314].rearrange("p (k o) -> p k o", k=TAPS)
            v_ident = t_sb[:, 1314:1442]
            nc.sync.dma_start(v_wconv, wconv[:])
            nc.sync.dma_start(v_wmain, wmain[:])
            nc.sync.dma_start(v_ident, ident[:])
            t_sf = static.tile([128, 92], F32)      # ck|chv|pvec
            v_ck = t_sf[:, 0:27]
            v_chv = t_sf[:, 27:91]
            v_pvec = t_sf[:, 91:92]
            nc.sync.dma_start(v_ck, ck[:])
            nc.sync.dma_start(v_chv, chv[:])
            nc.sync.dma_start(v_pvec, pvec[:])

            x2ap = x2[:]
            x2ov = bass.AP(tensor=x2ap.tensor, offset=x2ap.offset,
                           ap=[[128, Hp * Wp], [1, ELEM]])

            # per-block state passed from prep stage to compute stage
            state = {}

            def prep_block(blk):
                """conv + fields + idx wrap + gather issue for one block."""
                h0 = blk * HB
                # packed per-block bf16 tile: off | wc | i16
                t_fa = flda.tile([128, 576], BF16, tag="fa")
                v_off = t_fa[:, 0:216].rearrange("p (h k) -> p h k", h=HB)
                v_wc = t_fa[:, 216:504].rearrange(
                    "p (h k x y) -> p h k x y", h=HB, k=TAPS, x=2)
                v_i16 = t_fa[:, 504:576].bitcast(I16).rearrange(
                    "p (k h) -> p k h", k=TAPS)
                # packed per-block f32 scratch
                t_fb = fldb.tile([128, 1296], F32, tag="fb")
                v_F = t_fb[:, 0:216].rearrange("p (h k) -> p h k", h=HB)
                v_Fm = t_fb[:, 216:360].rearrange("p (h k) -> p h k", h=HB)
                v_i32 = t_fb[:, 360:504].bitcast(I32).rearrange(
                    "p (h k) -> p h k", h=HB)
                v_fl = t_fb[:, 504:648].rearrange("p (h k) -> p h k", h=HB)
                v_fr = t_fb[:, 648:792].rearrange("p (h k) -> p h k", h=HB)
                v_mask = t_fb[:, 792:864].rearrange("p (h k) -> p h k", h=HB)
                v_w11 = t_fb[:, 864:936].rearrange("p (h k) -> p h k", h=HB)
                v_w01 = t_fb[:, 936:1008].rearrange("p (h k) -> p h k", h=HB)
                v_w10 = t_fb[:, 1008:1080].rearrange("p (h k) -> p h k", h=HB)
                v_omty = t_fb[:, 1080:1152].rearrange("p (h k) -> p h k", h=HB)
                v_w00 = t_fb[:, 1152:1224].rearrange("p (h k) -> p h k", h=HB)
                v_lin = t_fb[:, 1224:1296].rearrange("p (h k) -> p h k", h=HB)

                # ---- conv: offsets+modulator, pos-major [128(w), h, 27] ----
                for h in range(HB):
                    hg = h0 + h
                    ps = pconv.tile([128, 27], F32, tag="pc")
                    for j, grp in enumerate(GROUPS):
                        kdim = 64 if len(grp) == 1 else 128
                        o = _conv_off(grp[0], hg)
                        nc.tensor.matmul(
                            ps[:], t_xab[:kdim, o:o + W],
                            v_wconv[:kdim, j, :],
                            start=(j == 0), stop=(j == len(GROUPS) - 1))
                    nc.scalar.copy(v_off[:, h, :], ps[:])

                # ---- fields ----
                # F = off + ck[k] (+ h for y cols, + p for x cols)
                nc.vector.tensor_tensor(
                    v_F, v_off,
                    bass.AP(tensor=v_ck.tensor, offset=v_ck.offset,
                            ap=[list(v_ck.ap[0]), [0, HB], [1, 27]]),
                    ALU.add)
                chs = v_chv[:, h0:h0 + HB]
                nc.vector.tensor_tensor(
                    v_F[:, :, 0:9], v_F[:, :, 0:9],
                    bass.AP(tensor=chs.tensor, offset=chs.offset,
                            ap=[list(chs.ap[0]), [1, HB], [0, 9]]),
                    ALU.add)
                nc.vector.tensor_scalar(v_F[:, :, 9:18], v_F[:, :, 9:18],
                                        v_pvec, None, ALU.add)
                nc.vector.tensor_scalar(v_Fm, v_F[:, :, 0:18], _FLOOR_BIAS,
                                        None, ALU.add)
                nc.vector.tensor_copy(v_i32, v_Fm)
                nc.vector.tensor_copy(v_fl, v_i32)
                nc.vector.tensor_tensor(v_fr, v_F[:, :, 0:18], v_fl,
                                        ALU.subtract)
                ty = v_fr[:, :, 0:9]
                tx = v_fr[:, :, 9:18]
                nc.scalar.activation(v_mask, v_F[:, :, 18:27], ACTF.Sigmoid)
                nc.vector.tensor_tensor(v_w11, ty, tx, ALU.mult)
                nc.vector.tensor_tensor(v_w01, tx, v_w11, ALU.subtract)
                nc.vector.tensor_tensor(v_w10, ty, v_w11, ALU.subtract)
                nc.vector.tensor_scalar(v_omty, ty, -1.0, 1.0,
                                        ALU.mult, ALU.add)
                nc.vector.tensor_tensor(v_w00, v_omty, v_w01, ALU.subtract)
                # wcomb[p, h, k, xc, yc] bf16, mask folded
                for (xc, yc, tw) in ((0, 0, v_w00), (0, 1, v_w10),
                                     (1, 0, v_w01), (1, 1, v_w11)):
                    nc.vector.tensor_tensor(v_wc[:, :, :, xc, yc], tw,
                                            v_mask, ALU.mult)
                # linear idx = y0*Wp + x0 (f32 exact) -> int16, free (k, h)
                nc.vector.tensor_scalar(v_lin, v_fl[:, :, 0:9], float(Wp),
                                        None, ALU.mult)
                nc.vector.tensor_tensor(v_lin, v_lin, v_fl[:, :, 9:18],
                                        ALU.add)
                nc.vector.tensor_copy(v_i16.rearrange("p k h -> p h k"),
                                      v_lin)

                # ---- idx wrap: [p, (k,h)] -> [r (x8), (k,h,q)] ----
                # bounce to DRAM p-major (contiguous per partition), read
                # back per 16-partition group with (q,k,h) order (144B
                # granules), then DVE-reorder free dims to (k,h,q).
                base = idxs_dram[blk % 2]
                nc.sync.dma_start(base, v_i16.rearrange("p k h -> p (k h)"))
                t_q = idxp.tile([128, KH * 8], I16, tag="iq")
                for g in range(8):
                    src_view = bass.AP(
                        tensor=base.tensor, offset=base.offset,
                        ap=[[KH, 16], [16 * KH, 8], [1, KH]])
                    nc.sync.dma_start(t_q[16 * g:16 * (g + 1)], src_view)
                t_idxw = idxp.tile([128, KH * 8], I16, tag="ix")
                nc.vector.tensor_copy(
                    t_idxw[:].rearrange("p (k h q) -> p k h q", h=HB, q=8),
                    t_q[:].rearrange("p (q k h) -> p k h q", q=8, k=TAPS))

                # ---- gather: 4 SWDGE queues (one per Q7 core pair) ----
                t_v = vp.tile([128, TAPS, HB, ELEM], BF16, tag="v")
                t_v_flat = t_v[:].rearrange("p k h e -> p (k h) e")
                per_q = KH // NQ                   # 18 dst slots of 128 idx
                for qi in range(NQ):
                    nidx_q = per_q * 128
                    nc.gpsimd.dma_gather(
                        t_v_flat[:, qi * per_q:(qi + 1) * per_q, :], x2ov,
                        t_idxw[:, qi * per_q * 8:(qi + 1) * per_q * 8],
                        nidx_q, nidx_q, ELEM, elem_step=128,
                        single_packet=False, queue_num=qi)
                state[blk] = (t_v, v_wc)

            def compute_block(blk):
                """combine + transpose + einsum + output for one block."""
                t_v, v_wc = state.pop(blk)
                t_st = stp.tile([128, TAPS, HB, 128], BF16, tag="st")
                for k in range(TAPS):
                    # combine: U = V * broadcast(wc), in place.
                    # per (tap, xc): ISA allows only 3 free dims
                    for xc in range(2):
                        vv = t_v[:, k, :, xc * 128:(xc + 1) * 128]
                        wv = bass.AP(
                            tensor=v_wc.tensor,
                            offset=v_wc.offset + k * 4 + xc * 2,
                            ap=[list(v_wc.ap[0]), [TAPS * 4, HB],
                                [0, 64], [1, 2]])
                        nc.vector.tensor_tensor(vv, vv, wv, ALU.mult)
                    # x-corner sum into the x0 half (DVE; PSUM cannot
                    # accumulate bf16 transposes on hw)
                    nc.vector.tensor_tensor(
                        t_v[:, k, :, 0:128], t_v[:, k, :, 0:128],
                        t_v[:, k, :, 128:256], ALU.add)
                    # transpose into psum; rows become (c, yc);
                    # y-corner sum happens in einsum (duplicated W rows)
                    vap = t_v[:]
                    pt = ptac.tile([128, HB, 128], BF16, tag="pt")
                    for h in range(HB):
                        uap = bass.AP(
                            tensor=vap.tensor,
                            offset=vap.offset + k * HB * ELEM + h * ELEM,
                            ap=[list(vap.ap[0]), [1, 128]])
                        nc.tensor.matmul(
                            pt[:, h, :], uap, v_ident,
                            start=True, stop=True, is_transpose=True)
                    nc.scalar.copy(t_st[:, k], pt[:])

                # ---- einsum: out[o, pos] += wmain_k^T @ S_T_k ----
                # tap weights stationary: load each wmain_j once, stream
                # both 512-pos chunks into two PSUM accumulation groups
                pos = [pout.tile([128, 512], F32, tag="po", name=f"po{i}")
                       for i in range(NPOS // 512)]
                for j in range(TAPS):
                    stv = t_st[:, j].rearrange("p h w -> p (h w)")
                    for cc in range(NPOS // 512):
                        nc.tensor.matmul(
                            pos[cc][:], v_wmain[:, j, :],
                            stv[:, cc * 512:(cc + 1) * 512],
                            start=(j == 0), stop=(j == TAPS - 1))
                for cc in range(NPOS // 512):
                    t_out = outp.tile([128, 512], F32, tag="out")
                    nc.scalar.copy(t_out[:], pos[cc][:])
                    base_o = blk * NPOS + cc * 512
                    nc.sync.dma_start(
                        out[:, base_o:base_o + 512], t_out[:])

            # software pipeline: block b+1's input prep (conv/fields/idx/
            # gather issue) is emitted BEFORE block b's compute so the
            # next gather's descriptor generation overlaps this block's
            # combine/transpose/einsum.
            prep_block(0)
            for blk in range(NBLK):
                if blk + 1 < NBLK:
                    prep_block(blk + 1)
                compute_block(blk)

    nc.finalize()
    return nc


# ---------------- host side ----------------

def prep_core_inputs(x, w_off, b_off, w_mod, b_mod, w_reg, s):
    """Build device input dict for shard s (image s//2, rows 64*(s%2)+)."""
    b, half = divmod(s, 2)
    r0 = half * Hs
    xb = np.asarray(x[b], dtype=np.float32)           # [C, H, W]

    # conv input, channel-major padded [64, 66, 130] bf16; second partition
    # half is the same data shifted by +1 element (for pair tap groups)
    xcm = np.zeros((Cin, CONV_H, CONV_W), np.float32)
    ylo = r0 - 1
    sylo, syhi = max(ylo, 0), min(ylo + CONV_H, H)
    xcm[:, sylo - ylo:syhi - ylo, 1:1 + W] = xb[:, sylo:syhi, :]
    xf = xcm.reshape(Cin, CONV_FLAT).astype(ml_dtypes.bfloat16)
    xab = np.zeros((128, CONV_FLAT), ml_dtypes.bfloat16)
    xab[:Cin] = xf
    xab[Cin:, :-1] = xf[:, 1:]

    # X2 gather source: [Hp, Wp, Cin, 2(yc)] -> flat [Hp*Wp (+1), 128]
    X2 = np.zeros((Hp, Wp, Cin, 2), np.float32)
    for yc in range(2):
        ylo = r0 - PADY + yc
        sylo, syhi = max(ylo, 0), min(ylo + Hp, H)
        X2[sylo - ylo:syhi - ylo, PADX:PADX + W, :, yc] = \
            xb[:, sylo:syhi, :].transpose(1, 2, 0)
    x2 = np.zeros((Hp * Wp + 1, Cin * 2), ml_dtypes.bfloat16)
    x2[:Hp * Wp] = X2.reshape(Hp * Wp, Cin * 2).astype(ml_dtypes.bfloat16)

    # decomposed consts: ck[*, 27] per-tap, chv[*, h]=h, pvec[p,1]=p
    ck = np.zeros((128, 27), np.float32)
    for k in range(TAPS):
        ky, kx = divmod(k, 3)
        ck[:, k] = ky + (PADY - 1) + float(b_off[2 * k])
        ck[:, 9 + k] = kx + (PADX - 1) + float(b_off[2 * k + 1])
        ck[:, 18 + k] = float(b_mod[k])
    chv = np.broadcast_to(np.arange(Hs, dtype=np.float32)[None, :],
                          (128, Hs)).copy()
    pvec = np.arange(128, dtype=np.float32).reshape(128, 1).copy()

    # conv weights [128, 6, 27]: group rows = [c of k0 | c of k1]
    wconv = np.zeros((128, 6, 27), np.float32)
    for j, grp in enumerate(GROUPS):
        for t, k in enumerate(grp):
            ky, kx = divmod(k, 3)
            for o in range(9):
                wconv[t * 64:(t + 1) * 64, j, o] = w_off[2 * o, :, ky, kx]
                wconv[t * 64:(t + 1) * 64, j, 9 + o] = w_off[2 * o + 1, :, ky, kx]
                wconv[t * 64:(t + 1) * 64, j, 18 + o] = w_mod[o, :, ky, kx]
    wconv = wconv.astype(ml_dtypes.bfloat16)

    # main weights [128, 9, 128]: chunk k rows = (c, yc) duplicated
    wmain = np.zeros((128, TAPS, 128), np.float32)
    for k in range(TAPS):
        ky, kx = divmod(k, 3)
        wt = 2.0 * w_reg[:, :, ky, kx].T        # [c, o]
        wmain[0::2, k, :] = wt
        wmain[1::2, k, :] = wt
    wmain = wmain.astype(ml_dtypes.bfloat16)

    return {"xab": xab, "x2": x2, "ck": ck, "chv": chv, "pvec": pvec,
            "wconv": wconv, "wmain": wmain,
            "ident": np.eye(128, dtype=ml_dtypes.bfloat16)}


_NC_CACHE = {}


def _run(x, w_off, b_off, w_mod, b_mod, w_reg, trace=False, **spmd_kwargs):
    from concourse.bass_utils import run_bass_kernel_spmd
    x = np.asarray(x); w_off = np.asarray(w_off); b_off = np.asarray(b_off)
    w_mod = np.asarray(w_mod); b_mod = np.asarray(b_mod)
    w_reg = np.asarray(w_reg)

    if "nc" not in _NC_CACHE:
        _NC_CACHE["nc"] = build_nc()
    nc = _NC_CACHE["nc"]

    in_maps = [prep_core_inputs(x, w_off, b_off, w_mod, b_mod, w_reg, s)
               for s in range(8)]
    res = run_bass_kernel_spmd(nc, in_maps, core_ids=list(range(8)),
                               trace=trace, **spmd_kwargs)
    results = res.results if hasattr(res, "results") else res

    out = np.zeros((B, Cout, H, W), np.float32)
    for s in range(8):
        b, half = divmod(s, 2)
        r0 = half * Hs
        out[b, :, r0:r0 + Hs, :] = \
            np.asarray(results[s]["out"]).reshape(Cout, Hs, W)
    return out, res


def kernel(x, w_off, b_off, w_mod, b_mod, w_reg):
    out, _ = _run(x, w_off, b_off, w_mod, b_mod, w_reg)
    return out
